# revision 21
# baseline (speedup 1.0000x reference)
"""GCN layer (message passing) Trainium2 Bass kernel, 8-core SPMD.

Math: out = norm * (segment_sum_dst(gather_src(norm * (h @ W)))) + b
Rewritten (matmul commutes with the linear aggregation):
    out = norm_dst * ((A @ (norm_src * h)) @ W) + b

Device strategy (per core c of 8; core owns dst rows [c*6250, (c+1)*6250)):
  - Host sorts edges by (core, dst-window, src-half); a window is 128
    consecutive dst rows (one PSUM partition block), src-half splits node
    ids at 25000 so gather indices fit int16.
  - gpsimd.dma_gather pulls h[src] rows (512B each) straight from HBM into
    SBUF tiles of [128 edges (partitions) x 128 feat], one DMA descriptor
    per edge. The gather is descriptor-rate-bound (~10ns/desc on HW), so:
    one call per (window, half) group on alternating SWDGE queues
    (num_swdge_queues=2, ~15% faster than one), single_packet=False, and
    the per-call descriptor count is statically trimmed to the max real
    edge count over cores (ceil-16) instead of the tile-padded count
    (~5% fewer descriptors). Skipped pad slots keep their memset/stale
    value and have mns==0, so they contribute nothing.
  - DVE/ACT scale-cast each tile by norm[src] (per-partition scalar) into
    bf16 messages; DVE builds a one-hot S tile [128 edges x 128 segs]
    (bf16) from iota==dstlocal. Gather buffers are 4 slots deep (s % 4)
    so desc-gen runs up to 4 windows ahead of the cast/matmul consumers.
  - PE accumulates aggT[F x 128 segs] = sum_tiles msgs^T @ S into PSUM
    (f32) per window; then aggT -> SBUF, out_w = aggT^T @ W (f32 matmul),
    scaled by norm_dst and biased on DVE, DMA'd out.
"""
import os
import sys
import numpy as np

for _p in ("/opt/trn_rl_repo",):
    if _p not in sys.path and os.path.isdir(_p):
        sys.path.insert(0, _p)

import ml_dtypes

import concourse.bacc as bacc
import concourse.bass as bass
import concourse.mybir as mybir
from concourse.alu_op_type import AluOpType
from concourse.library_config import mlp as mlp_library

BF16 = ml_dtypes.bfloat16

# ---------------- problem constants (hardcoded per contract) ----------------
N, F, E, C = 50000, 128, 800000, 8
R = N // C                      # 6250 dst rows per core
WIN = 128                       # segs per window (PSUM partition dim)
NW = (R + WIN - 1) // WIN       # 49 windows per core
HALF = 25000                    # src-half split (int16 gather indices)
SW = 1                          # windows per superwindow (gather-call granularity)
TILE = 128                      # edges per tile (PE contraction dim)
ACT_CAST_MOD = 8                # tile t cast on ACT if (t % 8) < ACT_CAST_FRAC
ACT_CAST_FRAC = 6
GCHUNK = 12                     # max tiles (x128 descriptors) per dma_gather call


# ---------------------------- host preprocessing ----------------------------

def host_prep(h, norm, W, b, src, dst):
    src = np.ascontiguousarray(np.asarray(src).astype(np.int64))
    dst = np.ascontiguousarray(np.asarray(dst).astype(np.int64))
    norm_f = np.asarray(norm, dtype=np.float32).reshape(-1)

    core = dst // R
    w_of = (dst % R) // WIN
    half = (src >= HALF).astype(np.int64)

    key = (core * NW + w_of) * 2 + half
    order = np.argsort(key, kind="stable")
    src_s, dst_s = src[order], dst[order]
    key_s = key[order]

    sizes = np.bincount(key_s, minlength=C * NW * 2).reshape(C, NW, 2)
    starts = np.zeros(C * NW * 2 + 1, dtype=np.int64)
    np.cumsum(sizes.reshape(-1), out=starts[1:])

    # tiles per (window, half): max over cores, both halves forced >= 1
    Tlo = np.maximum(1, -(-sizes[:, :, 0].max(axis=0) // TILE))
    Thi = np.maximum(1, -(-sizes[:, :, 1].max(axis=0) // TILE))

    n_sw = (NW + SW - 1) // SW
    # per-sw structure
    sw_windows = [list(range(s * SW, min((s + 1) * SW, NW))) for s in range(n_sw)]

    # global tile list: per sw, all lo groups (w asc) then all hi groups
    tile_window, tile_half = [], []
    call_list = []  # (sw, half, tile_start, n_tiles_call)
    for s in range(n_sw):
        for hf in (0, 1):
            t0 = len(tile_window)
            for w in sw_windows[s]:
                T = int((Tlo if hf == 0 else Thi)[w])
                tile_window += [w] * T
                tile_half += [hf] * T
            call_list.append((s, hf, t0, len(tile_window) - t0))
    tile_window = np.asarray(tile_window)
    tile_half = np.asarray(tile_half)
    n_tiles = len(tile_window)

    # per-window first/last tile (for matmul start/stop flags)
    first_tile = {}
    last_tile = {}
    for t, w in enumerate(tile_window):
        w = int(w)
        first_tile.setdefault(w, t)
        last_tile[w] = t

    # tiles per sw (buffer sizing) & tile offset within sw
    sw_tile_start = {}
    tile_off_in_sw = np.zeros(n_tiles, dtype=np.int64)
    for s, hf, t0, ntc in call_list:
        sw_tile_start.setdefault(s, t0)
    maxT_sw = 0
    for s in range(n_sw):
        t0 = sw_tile_start[s]
        t1 = sw_tile_start[s + 1] if s + 1 in sw_tile_start else n_tiles
        maxT_sw = max(maxT_sw, t1 - t0)
        tile_off_in_sw[t0:t1] = np.arange(t1 - t0)

    # chunked gather-call layout (must mirror build_program's gpsimd loop)
    chunk_list = []  # (s, hf, t0, ntc, tt, nt)
    for s, hf, t0, ntc in call_list:
        for c0 in range(0, ntc, GCHUNK):
            nt = min(GCHUNK, ntc - c0)
            chunk_list.append((s, hf, t0, ntc, t0 + c0, nt))
    n_calls = len(chunk_list)

    pattern = dict(
        Tlo=Tlo, Thi=Thi, n_tiles=n_tiles, n_sw=n_sw, sw_windows=sw_windows,
        tile_window=tile_window, tile_half=tile_half, call_list=call_list,
        first_tile=first_tile, last_tile=last_tile, maxT_sw=maxT_sw,
        sw_tile_start=sw_tile_start, tile_off_in_sw=tile_off_in_sw,
        chunk_list=chunk_list, n_calls=n_calls,
    )

    # ---- per-core data ----
    cores = []
    W_np = np.asarray(W, dtype=np.float32)
    b_np = np.asarray(b, dtype=np.float32).reshape(-1)
    bias_tile = np.tile(b_np[None, :], (128, 1)).astype(np.float32)
    iota_bf = np.tile(np.arange(WIN, dtype=np.float32)[None, :], (128, 1)).astype(BF16)
    h_np = np.ascontiguousarray(np.asarray(h, dtype=np.float32))

    call_real = [dict() for _ in range(n_calls)]
    per_core_gidx_flat = []
    for c in range(C):
        gidx_flat = np.full(n_tiles * TILE, -1, dtype=np.int16)
        mdst_flat = np.zeros(n_tiles * TILE, dtype=np.float32)
        mns_flat = np.zeros(n_tiles * TILE, dtype=np.float32)
        group_count = np.zeros(n_tiles * TILE, dtype=bool)  # real-edge mask
        for s, hf, t0, ntc in call_list:
            pos = t0 * TILE
            for w in sw_windows[s]:
                g = (c * NW + w) * 2 + hf
                st, en = int(starts[g]), int(starts[g + 1])
                n = en - st
                T = int((Tlo if hf == 0 else Thi)[w])
                sl = slice(pos, pos + n)
                gsrc = src_s[st:en]
                gidx_flat[sl] = (gsrc - hf * HALF).astype(np.int16)
                mdst_flat[sl] = (dst_s[st:en] - c * R - w * WIN).astype(np.float32)
                mns_flat[sl] = norm_f[gsrc]
                group_count[sl] = True
                pos += T * TILE

        # Per-call real-edge counts; the static (core-independent) trimmed
        # descriptor count is the max over cores, ceil-16. Descriptors beyond
        # it are never generated; slots between a core's real count and the
        # static count gather row 0 (their mns is 0: zero contribution).
        for ci, (s, hf, t0, ntc, tt, nt) in enumerate(chunk_list):
            n_real = int(group_count[tt * TILE:(tt + nt) * TILE].sum())
            call_real[ci][c] = n_real

        per_core_gidx_flat.append((gidx_flat, mdst_flat, mns_flat))

    # static trimmed count per call + effective tile count per call
    nidx_static = np.zeros(n_calls, dtype=np.int64)
    for ci, (s, hf, t0, ntc, tt, nt) in enumerate(chunk_list):
        m = max(call_real[ci].values())
        nidx_static[ci] = min(max(-(-m // 16) * 16, 16), nt * TILE)
    pattern["nidx_static"] = nidx_static

    for c in range(C):
        gidx_flat, mdst_flat, mns_flat = per_core_gidx_flat[c]
        # row-0 filler gathers between this core's real count and the static
        # trimmed count (sim contract: reg count == #non-negative idxs)
        for ci, (s, hf, t0, ntc, tt, nt) in enumerate(chunk_list):
            lo = tt * TILE + call_real[ci][c]
            hi = tt * TILE + int(nidx_static[ci])
            if hi > lo:
                gidx_flat[lo:hi] = 0

        # gather idx wrapped layout per call: [16, n/16] blocks, tiled x8
        blocks = []
        for s, hf, t0, ntc in call_list:
            fl = gidx_flat[t0 * TILE:(t0 + ntc) * TILE]
            blocks.append(fl.reshape(-1, 16).T)           # [16, ntc*8]
        gidx_wrapped = np.tile(np.concatenate(blocks, axis=1), (8, 1))  # [128, n_tiles*8]

        mdst_t = mdst_flat.reshape(n_tiles, TILE).T.copy()  # [128, n_tiles]
        mns_t = mns_flat.reshape(n_tiles, TILE).T.copy()

        nd = np.zeros((WIN, NW), dtype=np.float32)
        for w in range(NW):
            lo = c * R + w * WIN
            hi = min(lo + WIN, (c + 1) * R)
            nd[: hi - lo, w] = norm_f[lo:hi]

        cores.append({
            "h": h_np,
            "gidx": np.ascontiguousarray(gidx_wrapped),
            "mdst": np.ascontiguousarray(mdst_t),
            "mns": np.ascontiguousarray(mns_t),
            "norm_dst": np.ascontiguousarray(nd),
            "Wmat": W_np,
            "bias_tile": bias_tile,
            "iota_bf": iota_bf,
        })
    return cores, pattern


# ----------------------------- device program -------------------------------

def build_program(pat):
    n_tiles = pat["n_tiles"]
    n_sw = pat["n_sw"]
    maxT = pat["maxT_sw"]
    tile_window = pat["tile_window"]
    first_tile, last_tile = pat["first_tile"], pat["last_tile"]
    call_list = pat["call_list"]
    sw_windows = pat["sw_windows"]
    sw_tile_start = pat["sw_tile_start"]
    tile_off = pat["tile_off_in_sw"]

    def sw_tiles(s):
        t0 = sw_tile_start[s]
        t1 = sw_tile_start[s + 1] if s + 1 in sw_tile_start else n_tiles
        return list(range(t0, t1))

    def is_act_cast(t):
        return (t % ACT_CAST_MOD) < ACT_CAST_FRAC

    # ---- schedules & counters (for cross-engine wait targets) ----
    # PE op order: per sw: tiles, then W-matmuls of previous sw's windows.
    pe_count_after = {}   # key: ("tile", t) or ("wmm", w) -> pe_c value after op
    cnt = 0
    for s in range(n_sw + 1):
        if s < n_sw:
            for t in sw_tiles(s):
                cnt += 1
                pe_count_after[("tile", t)] = cnt
        if s >= 1:
            for w in sw_windows[s - 1]:
                cnt += 1
                pe_count_after[("wmm", w)] = cnt
    pe_total = cnt
    pe_after_tiles_of_sw = {}
    for s in range(n_sw):
        pe_after_tiles_of_sw[s] = pe_count_after[("tile", sw_tiles(s)[-1])]

    # cast counters
    acast_pos, vcast_pos = {}, {}
    na = nv = 0
    for t in range(n_tiles):
        if is_act_cast(t):
            na += 1
            acast_pos[t] = na
        else:
            nv += 1
            vcast_pos[t] = nv
    acast_through_sw = {}
    vcast_through_sw = {}
    ca = cv = 0
    for s in range(n_sw):
        for t in sw_tiles(s):
            if is_act_cast(t):
                ca += 1
            else:
                cv += 1
        acast_through_sw[s] = ca
        vcast_through_sw[s] = cv

    dt = mybir.dt
    nc = bacc.Bacc("TRN2", debug=False, num_swdge_queues=2,
                   dynamic_dma_scratch_size=32768)

    n_calls = pat["n_calls"]
    chunk_list = pat["chunk_list"]
    nidx_static = pat["nidx_static"]
    h_d = nc.dram_tensor("h", [N, F], dt.float32, kind="ExternalInput")
    gidx_d = nc.dram_tensor("gidx", [128, n_tiles * 8], dt.int16, kind="ExternalInput")
    mdst_d = nc.dram_tensor("mdst", [128, n_tiles], dt.float32, kind="ExternalInput")
    mns_d = nc.dram_tensor("mns", [128, n_tiles], dt.float32, kind="ExternalInput")
    nd_d = nc.dram_tensor("norm_dst", [WIN, NW], dt.float32, kind="ExternalInput")
    W_d = nc.dram_tensor("Wmat", [F, F], dt.float32, kind="ExternalInput")
    bias_d = nc.dram_tensor("bias_tile", [128, F], dt.float32, kind="ExternalInput")
    iota_d = nc.dram_tensor("iota_bf", [128, WIN], dt.bfloat16, kind="ExternalInput")
    out_d = nc.dram_tensor("out", [NW * WIN, F], dt.float32, kind="ExternalOutput")

    sb_gidx = nc.alloc_sbuf_tensor("sb_gidx", [128, n_tiles * 8], dt.int16)
    sb_mdst = nc.alloc_sbuf_tensor("sb_mdst", [128, n_tiles], dt.float32)
    sb_mns = nc.alloc_sbuf_tensor("sb_mns", [128, n_tiles], dt.float32)
    sb_nd = nc.alloc_sbuf_tensor("sb_nd", [WIN, NW], dt.float32)
    sb_W = nc.alloc_sbuf_tensor("sb_W", [F, F], dt.float32)
    sb_bias = nc.alloc_sbuf_tensor("sb_bias", [128, F], dt.float32)
    sb_iota = nc.alloc_sbuf_tensor("sb_iota", [128, WIN], dt.bfloat16)

    NSLOT = 4
    gbuf = nc.alloc_sbuf_tensor("gbuf", [128, NSLOT, maxT, TILE], dt.float32)
    mbuf = nc.alloc_sbuf_tensor("mbuf", [128, NSLOT, maxT, TILE], dt.bfloat16)
    sbuf_S = nc.alloc_sbuf_tensor("sbuf_S", [128, NSLOT, maxT, WIN], dt.bfloat16)
    aggTs = nc.alloc_sbuf_tensor("aggTs", [F, 2, WIN], dt.float32)
    outsb = nc.alloc_sbuf_tensor("outsb", [WIN, 2, F], dt.float32)

    # PSUM: agg slots (w%4) in banks 0-1... slot stride = 512 f32 = 2KB = 1 bank
    ps_agg = nc.alloc_psum_tensor("ps_agg", [128, 4, 512], dt.float32)
    ps_out = nc.alloc_psum_tensor("ps_out", [128, 2, 512], dt.float32)

    ld = nc.alloc_semaphore("ld")
    gld = nc.alloc_semaphore("gld")
    gld2 = nc.alloc_semaphore("gld2")
    msem = nc.alloc_semaphore("msem")
    gsem = [[nc.alloc_semaphore(f"gsem{p}{q}") for q in (0, 1)]
            for p in range(4)]
    mready_a = nc.alloc_semaphore("mready_a")
    mready_v = nc.alloc_semaphore("mready_v")
    sready = nc.alloc_semaphore("sready")
    pe_c = nc.alloc_semaphore("pe_c")
    aggc = nc.alloc_semaphore("aggc")
    dvsc = nc.alloc_semaphore("dvsc")
    osem = [nc.alloc_semaphore("osem0"), nc.alloc_semaphore("osem1")]

    N_LOADS = 6
    # gidx column where sw 8's tiles start (16-wrapped cols = tile*8)
    _t8 = sw_tile_start.get(8, n_tiles)
    GIDX_SPLIT = _t8 * 8

    # Gather calls alternate between the 2 SWDGE queues (parallel desc-gen
    # rings). gsem[q] counts completions per queue; consumers of sw s wait
    # both queues' cumulative totals through s (per-queue completion is FIFO).
    # Calls alternate SWDGE queues; completion sems are per (sw parity,
    # queue) so consumers never wait on intermediate values of a sem that is
    # still being incremented (same proof structure as the baseline's
    # per-parity gsem: sw s's calls are issued only after sw s-2's casts).
    call_queue = []
    qcum = {}  # (parity, q) -> calls so far
    gsem_target = {}  # s -> (q0_units, q1_units) cumulative for parity s%2
    for _k, (s, hf, t0, ntc, tt, nt) in enumerate(chunk_list):
        q = _k % 2
        call_queue.append(q)
        qcum[(s % 4, q)] = qcum.get((s % 4, q), 0) + 1
        gsem_target[s] = (16 * qcum.get((s % 4, 0), 0),
                          16 * qcum.get((s % 4, 1), 0))

    with nc.Block() as block:

        @block.sync
        def _(sync: bass.BassEngine):
            sync.dma_start(sb_gidx[:, :GIDX_SPLIT],
                           gidx_d[:, :GIDX_SPLIT]).then_inc(gld, 16)
            sync.dma_start(sb_gidx[:, GIDX_SPLIT:],
                           gidx_d[:, GIDX_SPLIT:]).then_inc(gld2, 16)
            sync.dma_start(sb_mdst[:, :], mdst_d[:, :]).then_inc(ld, 16)
            sync.dma_start(sb_mns[:, :], mns_d[:, :]).then_inc(ld, 16)
            sync.dma_start(sb_nd[:, :], nd_d[:, :]).then_inc(ld, 16)
            sync.dma_start(sb_W[:, :], W_d[:, :]).then_inc(ld, 16)
            sync.dma_start(sb_bias[:, :], bias_d[:, :]).then_inc(ld, 16)
            sync.dma_start(sb_iota[:, :], iota_d[:, :]).then_inc(ld, 16)
            for w in range(NW):
                sync.wait_ge(dvsc, w + 1)
                sync.dma_start(
                    out_d[w * WIN:(w + 1) * WIN, :], outsb[:, w % 2, :]
                ).then_inc(osem[w % 2], 16)

        @block.gpsimd
        def _(gp: bass.BassGpSimd):
            gp.load_library(mlp_library)
            gp.wait_ge(gld, 16)
            gp.wait_ge(msem, 1)  # gbuf memset done before first gather lands
            seen_s = set()
            waited_g2 = False
            for k, (s, hf, t0, ntc, tt, nt) in enumerate(chunk_list):
                if s not in seen_s:
                    seen_s.add(s)
                    if s >= 4:
                        # gbuf slot s%4 free once casts of sw s-4 consumed it
                        gp.wait_ge(mready_a, acast_through_sw[s - 4])
                        gp.wait_ge(mready_v, vcast_through_sw[s - 4])
                # Statically trimmed descriptor count (max real count over
                # cores, ceil-16). Tiles beyond ceil(n_trim/128) are never
                # gathered: their gbuf stays at the memset value and mns is 0
                # there, so they contribute nothing.
                n_trim = int(nidx_static[k])
                nt_eff = -(-n_trim // TILE)
                off = int(tile_off[tt])
                q = call_queue[k]
                if not waited_g2 and tt * 8 + n_trim // 16 > GIDX_SPLIT:
                    gp.wait_ge(gld2, 16)
                    waited_g2 = True
                gp.dma_gather(
                    gbuf[:, s % 4, off:off + nt_eff, :],
                    h_d[hf * HALF:hf * HALF + HALF, :],
                    sb_gidx[:, tt * 8:tt * 8 + n_trim // 16],
                    n_trim,
                    n_trim,
                    F,
                    queue_num=q,
                    single_packet=False,
                ).then_inc(gsem[s % 4][q], 16)

        @block.tensor
        def _(pe):
            pe.wait_ge(ld, 16 * N_LOADS)
            for s in range(n_sw + 1):
                if s < n_sw:
                    for t in sw_tiles(s):
                        w = int(tile_window[t])
                        if first_tile[w] == t and w >= 4:
                            pe.wait_ge(aggc, w - 3)
                        if is_act_cast(t):
                            pe.wait_ge(mready_a, acast_pos[t])
                        else:
                            pe.wait_ge(mready_v, vcast_pos[t])
                        pe.wait_ge(sready, t + 1)
                        j = int(tile_off[t])
                        pe.matmul(
                            ps_agg[:, w % 4, 0:WIN],
                            mbuf[:, s % 4, j, :],
                            sbuf_S[:, s % 4, j, :],
                            start=(first_tile[w] == t),
                            stop=(last_tile[w] == t),
                        ).then_inc(pe_c)
                if s >= 1:
                    for w in sw_windows[s - 1]:
                        pe.wait_ge(aggc, w + 1)
                        if w >= 2:
                            pe.wait_ge(dvsc, w - 1)
                        pe.matmul(
                            ps_out[:, w % 2, 0:F],
                            aggTs[:, w % 2, :],
                            sb_W[:, :],
                            start=True,
                            stop=True,
                        ).then_inc(pe_c)

        @block.scalar
        def _(act):
            act.wait_ge(msem, 1)
            act.wait_ge(ld, 16 * N_LOADS)
            for s in range(n_sw + 1):
                if s < n_sw:
                    if s >= 4:
                        act.wait_ge(pe_c, pe_after_tiles_of_sw[s - 4])
                    waited_g = False
                    for t in sw_tiles(s):
                        if not is_act_cast(t):
                            continue
                        if not waited_g:
                            act.wait_ge(gsem[s % 4][0], gsem_target[s][0])
                            act.wait_ge(gsem[s % 4][1], gsem_target[s][1])
                            waited_g = True
                        j = int(tile_off[t])
                        act.activation(
                            mbuf[:, s % 4, j, :],
                            gbuf[:, s % 4, j, :],
                            mybir.ActivationFunctionType.Copy,
                            scale=sb_mns[:, t:t + 1],
                        ).then_inc(mready_a)
                if s >= 1:
                    for w in sw_windows[s - 1]:
                        tgt = pe_count_after[("tile", last_tile[w])]
                        if w >= 2:
                            tgt = max(tgt, pe_count_after[("wmm", w - 2)])
                        act.wait_ge(pe_c, tgt)
                        act.activation(
                            aggTs[:, w % 2, :],
                            ps_agg[:, w % 4, 0:WIN],
                            mybir.ActivationFunctionType.Copy,
                        ).then_inc(aggc)

        @block.vector
        def _(dve):
            # stale gbuf slots (descriptor-trimmed calls skip them) must be
            # finite: 0 * mns(=0) = 0. One-time ~5us.
            dve.memset(gbuf[:, :, :, :], 0.0).then_inc(msem, 1)
            dve.wait_ge(ld, 16 * N_LOADS)
            for s in range(n_sw + 1):
                if s < n_sw:
                    if s >= 4:
                        dve.wait_ge(pe_c, pe_after_tiles_of_sw[s - 4])
                    waited_g = False
                    for t in sw_tiles(s):
                        j = int(tile_off[t])
                        if not is_act_cast(t):
                            if not waited_g:
                                dve.wait_ge(gsem[s % 4][0], gsem_target[s][0])
                                dve.wait_ge(gsem[s % 4][1], gsem_target[s][1])
                                waited_g = True
                            dve.tensor_scalar(
                                mbuf[:, s % 4, j, :],
                                gbuf[:, s % 4, j, :],
                                sb_mns[:, t:t + 1],
                                None,
                                AluOpType.mult,
                            ).then_inc(mready_v)
                        dve.tensor_scalar(
                            sbuf_S[:, s % 4, j, :],
                            sb_iota[:, :],
                            sb_mdst[:, t:t + 1],
                            None,
                            AluOpType.is_equal,
                        ).then_inc(sready)
                if s >= 1:
                    for w in sw_windows[s - 1]:
                        dve.wait_ge(pe_c, pe_count_after[("wmm", w)])
                        if w >= 2:
                            dve.wait_ge(osem[w % 2], 16 * (w // 2))
                        # outsb = (ps_out * norm_dst) + bias, one fused DVE op
                        dve.scalar_tensor_tensor(
                            outsb[:, w % 2, :],
                            ps_out[:, w % 2, 0:F],
                            sb_nd[:, w:w + 1],
                            sb_bias[:, :],
                            AluOpType.mult,
                            AluOpType.add,
                        ).then_inc(dvsc)

    nc.compile()
    return nc


# ------------------------------- entry point --------------------------------

def kernel(h, norm, W, b, src, dst):
    cores, pat = host_prep(h, norm, W, b, src, dst)
    nc = build_program(pat)

    from concourse.bass_utils import run_bass_kernel_spmd
    res = run_bass_kernel_spmd(nc, cores, core_ids=list(range(C)))
    outs = [res.results[c]["out"][:R] for c in range(C)]
    return np.ascontiguousarray(np.concatenate(outs, axis=0).astype(np.float32))

